# revision 28
# baseline (speedup 1.0000x reference)
"""Causal MQA self-attention (B=4, T=2048, D=1024, H=16 q-heads, 1 shared KV head)
on 8 TRN2 NeuronCores.

Sharding: core c = (b, g) with b = c // 2 (batch), g = c % 2 (head group of 8
query heads). Tensor-parallel on c_attn q-output columns and c_proj rows;
shared K/V computed per core from replicated wkv columns. Each core emits a
partial [T, D] projection output; the host sums the two head-group partials
per batch.

Per-core math (all matmuls fp32r, PE-friendly layouts):
  qkvT = W.T @ x.T                      (x fed pre-transposed as xT [D, T])
  S^T[tk, tq] = k q^T (K=64)            (scores transposed: softmax dim on
                                         partitions so PV contracts on it)
  P^T = exp(S^T / 8) with causal block-skip + triangular mask on diagonal
  [y^T; sums] = [v | 1].T @ P^T         (row-sums ride along as output row 64)
  y_norm = y / sums                     (done in a small transposed layout)
  out_partial = y_norm.T @ wp_slice     ([T, D], accumulated over head dims)

Engines cannot move data across partitions, so: K is duplicated into both
partition halves (odd heads' q slices live at partition 64), V stays in the
upper half and transposes against the identity's lower-right block, and head
pairs share one [128, 128] normalized-y tile that transposes directly into
the paired yT layout the projection needs.
"""

import numpy as np

import concourse.bacc as bacc
import concourse.tile as tile
from concourse import mybir
from concourse import bass_utils

F32 = mybir.dt.float32
F32R = mybir.dt.float32r
BF16 = mybir.dt.bfloat16
EXP = mybir.ActivationFunctionType.Exp

B, T, D, H, HD = 4, 2048, 1024, 16, 64
N_CORES = 8
HPC = H // 2          # 8 query heads per core
GQ = HPC * HD         # 512 q columns per core
QCH = 512             # Tq chunk width
KCH = 128             # Tk chunk width
NTQ = T // QCH        # 4
NT128 = T // 128      # 16
ND = D // 128         # 8

_CACHED_NC = None

DEFAULT_OPTS = {
    "psA_bufs": 3,   # scores/qkv/proj PSUM tiles
    "psY_bufs": 2,   # PV-accumulator PSUM tiles
    "psT_bufs": 2,   # transpose PSUM tiles
    "ptf_bufs": 4,   # fully-live probability tiles
    "xt_bufs": 10,   # streamed xT tiles
    "av_trim": True,  # trim PV matmul to live columns on diagonal-band tiles
    # timing-only debug switches (produce wrong numerics):
    "dbg_no_exp": False,   # DVE copy instead of ACT exp
    "dbg_no_norm": False,  # skip the y-normalize/transpose chain
    "dbg_no_proj": False,  # skip the projection phase
    "dbg_no_qkv": False,   # skip the qkv matmuls
    "loop_n": 0,           # >0: wrap the whole pipeline in a HW loop (timing)
    "mmdt": "bf16",        # matmul operand dtype: "bf16" (2x PE) or "f32r"
    "wide_exp": True,      # pair fully-live tiles into [128,1024] PSUM + one exp
}


def build_program(opts=None, cache=True):
    """Build (and cache) the single-core Bass program shared by all 8 cores."""
    global _CACHED_NC
    if cache and opts is None and _CACHED_NC is not None:
        return _CACHED_NC
    o = dict(DEFAULT_OPTS)
    if opts:
        o.update(opts)
    if o["wide_exp"] and "psA_bufs" not in (opts or {}):
        o["psA_bufs"] = 2

    MDT = BF16 if o["mmdt"] == "bf16" else F32R

    nc = bacc.Bacc(
        "TRN2", target_bir_lowering=False, debug=False, num_devices=N_CORES
    )
    xT_ap = nc.dram_tensor("xT", [D, T], MDT, kind="ExternalInput").ap()
    wq_ap = nc.dram_tensor("wq", [D, GQ], MDT, kind="ExternalInput").ap()
    wkv_ap = nc.dram_tensor("wkv", [D, 2 * HD], MDT, kind="ExternalInput").ap()
    wp_ap = nc.dram_tensor("wp", [GQ, D], MDT, kind="ExternalInput").ap()
    mask_ap = nc.dram_tensor("mask", [128, 128], MDT, kind="ExternalInput").ap()
    ident_ap = nc.dram_tensor("ident", [128, 128], F32, kind="ExternalInput").ap()
    out_ap = nc.dram_tensor("out", [T, D], F32, kind="ExternalOutput").ap()

    with tile.TileContext(nc) as tc:
        with (
            tc.tile_pool(name="sb", bufs=1) as sb,
            tc.tile_pool(name="psA", bufs=o["psA_bufs"], space="PSUM") as psA,
            tc.tile_pool(name="psY", bufs=o["psY_bufs"], space="PSUM") as psY,
            tc.tile_pool(name="psT", bufs=o["psT_bufs"], space="PSUM") as psT,
        ):
            # --- persistent SBUF tensors ---
            wq_sb = [
                sb.tile([128, GQ], MDT, name=f"wq{d}", tag=f"wq{d}")
                for d in range(ND)
            ]
            wkv_sb = [
                sb.tile([128, 2 * HD], MDT, name=f"wkv{d}", tag=f"wkv{d}")
                for d in range(ND)
            ]
            wp_sb = [
                sb.tile([128, D], MDT, name=f"wp{i}", tag=f"wp{i}")
                for i in range(4)
            ]
            qT_sb = [
                sb.tile([128, T], MDT, name=f"qT{m}", tag=f"qT{m}")
                for m in range(4)
            ]
            kT_sb = sb.tile([128, T], MDT, name="kT", tag="kT")
            vT_sb = sb.tile([128, T], F32, name="vT", tag="vT")  # rows 64:128
            # v chunks + ones column: [128, 65] per 128-row T chunk
            vx_sb = sb.tile([128, NT128 * 65], MDT, name="vx", tag="vx")
            yTn_sb = [
                sb.tile([128, T], MDT, name=f"yTn{i}", tag=f"yTn{i}")
                for i in range(4)
            ]
            mask_sb = sb.tile([128, 128], MDT, name="mask_sb", tag="mask")
            ident_sb = sb.tile([128, 128], F32, name="ident_sb", tag="ident")

            for d in range(ND):
                nc.sync.dma_start(wq_sb[d][:], wq_ap[d * 128:(d + 1) * 128, :])
                nc.sync.dma_start(wkv_sb[d][:], wkv_ap[d * 128:(d + 1) * 128, :])
            for i in range(4):
                nc.sync.dma_start(wp_sb[i][:], wp_ap[i * 128:(i + 1) * 128, :])
            nc.sync.dma_start(mask_sb[:], mask_ap[:])
            nc.sync.dma_start(ident_sb[:], ident_ap[:])
            for t in range(NT128):
                ones_ap = vx_sb[:, t * 65 + 64:t * 65 + 65]
                nc.vector.memset(ones_ap.bitcast(F32) if MDT == F32R else ones_ap, 1.0)

            def emit_qkv(t):
                """QKV projection for T-chunk t: writes qT/kT (both halves)/vx
                columns [t*512, (t+1)*512)."""
                xts = []
                for d in range(ND):
                    xt = sb.tile(
                        [128, QCH], MDT, name=f"xt_{t}_{d}", tag="xt",
                        bufs=o["xt_bufs"],
                    )
                    nc.sync.dma_start(
                        xt[:], xT_ap[d * 128:(d + 1) * 128, t * QCH:(t + 1) * QCH]
                    )
                    xts.append(xt)
                for m in range(5):
                    if o["dbg_no_qkv"]:
                        break
                    ps = psA.tile(
                        [128, QCH], F32, name=f"qkvps_{t}_{m}", tag="mm"
                    )
                    for d in range(ND):
                        lhsT = (
                            wq_sb[d][:, m * 128:(m + 1) * 128]
                            if m < 4
                            else wkv_sb[d][:]
                        )
                        nc.tensor.matmul(
                            ps[:], lhsT, xts[d][:],
                            start=(d == 0), stop=(d == ND - 1),
                        )
                    if m < 4:
                        nc.vector.tensor_copy(
                            qT_sb[m][:, t * QCH:(t + 1) * QCH], ps[:]
                        )
                    else:
                        nc.vector.tensor_copy(
                            kT_sb[0:HD, t * QCH:(t + 1) * QCH], ps[0:HD, :]
                        )
                        nc.vector.tensor_copy(
                            vT_sb[HD:128, t * QCH:(t + 1) * QCH], ps[HD:128, :]
                        )
                # duplicate this k chunk into the lower half (SBUF->SBUF DMA)
                nc.sync.dma_start(
                    kT_sb[HD:128, t * QCH:(t + 1) * QCH],
                    kT_sb[0:HD, t * QCH:(t + 1) * QCH],
                )
                # v chunks transposed into [Tk, 64] layout (+ ones col set)
                for tt in range(t * 4, t * 4 + 4):
                    vtp = psT.tile([128, 65], F32, name=f"vtp_{tt}", tag="tp")
                    nc.tensor.transpose(
                        vtp[0:128, 0:HD],
                        vT_sb[HD:128, tt * 128:(tt + 1) * 128],
                        ident_sb[HD:128, HD:128],
                    )
                    nc.vector.tensor_copy(
                        vx_sb[:, tt * 65:tt * 65 + HD], vtp[0:128, 0:HD]
                    )

            # --- probability tiles, grouped by diagonal-offset class.
            # Dead columns are zeroed once; exp only ever writes live columns,
            # so the PV matmul can always read the full 512-wide tile. ---
            ptf_w = 2 * QCH if o["wide_exp"] else QCH
            pt_full = [
                sb.tile([128, ptf_w], MDT, name=f"ptf{i}", tag=f"ptf{i}")
                for i in range(o["ptf_bufs"])
            ]
            pt_diag = {
                off: [
                    sb.tile(
                        [128, QCH], MDT, name=f"ptd{off}_{i}", tag=f"ptd{off}_{i}"
                    )
                    for i in range(2)
                ]
                for off in (0, 128, 256, 384)
            }
            for off in (128, 256, 384):
                for til in pt_diag[off]:
                    dead = til[:, 0:off]
                    nc.vector.memset(dead.bitcast(F32) if MDT == F32R else dead, 0.0)

            # --- phase 2: attention, head pairs share one yq2 tile per chunk ---
            cnt_full = 0
            cnt_diag = {0: 0, 128: 0, 256: 0, 384: 0}

            def emit_attn(qc):
                nonlocal cnt_full
                for m in range(4):  # head pair (2m, 2m+1)
                    yq2 = [
                        sb.tile(
                            [128, 128], F32, name=f"yq2_{m}_{qc}_{j}",
                            tag=f"yq2_{j}", bufs=2,
                        )
                        for j in range(4)
                    ]
                    for hh in range(2):
                        r = hh * HD
                        yps = psY.tile(
                            [65, QCH], F32, name=f"y_{m}_{hh}_{qc}", tag="y"
                        )
                        nkc = (qc + 1) * (QCH // KCH)
                        n_full = nkc - 4  # tiles strictly below the diag band

                        def emit_scores(sps, s_lo, s_hi, kc):
                            nc.tensor.matmul(
                                sps[:, s_lo:s_hi],
                                kT_sb[r:r + HD, kc * KCH:(kc + 1) * KCH],
                                qT_sb[m][
                                    r:r + HD,
                                    qc * QCH + (s_lo % QCH):
                                    qc * QCH + (s_lo % QCH) + (s_hi - s_lo),
                                ],
                                start=True, stop=True,
                            )

                        def emit_exp(pt, sps, lo, hi):
                            if o["dbg_no_exp"]:
                                nc.vector.tensor_copy(pt[:, lo:hi], sps[:, lo:hi])
                            else:
                                nc.scalar.activation(
                                    pt[:, lo:hi], sps[:, lo:hi], EXP, scale=0.125
                                )

                        kc = 0
                        first_av = True
                        if o["wide_exp"]:
                            while kc + 1 < n_full:  # full tiles, in pairs
                                pt = pt_full[cnt_full % o["ptf_bufs"]]
                                cnt_full += 1
                                sps = psA.tile(
                                    [128, 2 * QCH], F32,
                                    name=f"s_{m}_{hh}_{qc}_{kc}", tag="mm",
                                )
                                emit_scores(sps, 0, QCH, kc)
                                emit_scores(sps, QCH, 2 * QCH, kc + 1)
                                emit_exp(pt, sps, 0, 2 * QCH)
                                nc.tensor.matmul(
                                    yps[:], vx_sb[:, kc * 65:kc * 65 + 65],
                                    pt[:, 0:QCH], start=first_av, stop=False,
                                )
                                nc.tensor.matmul(
                                    yps[:],
                                    vx_sb[:, (kc + 1) * 65:(kc + 1) * 65 + 65],
                                    pt[:, QCH:2 * QCH], start=False, stop=False,
                                )
                                first_av = False
                                kc += 2
                        while kc < nkc:
                            off = kc * KCH - qc * QCH
                            if off < 0:  # leftover single full tile
                                pt = pt_full[cnt_full % o["ptf_bufs"]]
                                cnt_full += 1
                                mm_lo = exp_lo = 0
                                diag = False
                            else:  # diagonal band tile
                                pt = pt_diag[off][cnt_diag[off] % 2]
                                cnt_diag[off] += 1
                                exp_lo = off
                                mm_lo = 256 if off == 384 else off
                                diag = True
                            sps = psA.tile(
                                [128, QCH], F32, name=f"s_{m}_{hh}_{qc}_{kc}",
                                tag="mm",
                            )
                            emit_scores(sps, mm_lo, QCH, kc)
                            emit_exp(pt, sps, exp_lo, QCH)
                            if diag:
                                nc.vector.tensor_mul(
                                    pt[:, exp_lo:exp_lo + 128],
                                    pt[:, exp_lo:exp_lo + 128],
                                    mask_sb[:],
                                )
                            av_lo = (exp_lo if o["av_trim"] else 0) if not first_av else 0
                            nc.tensor.matmul(
                                yps[:, av_lo:QCH],
                                vx_sb[:, kc * 65:kc * 65 + 65],
                                pt[:, av_lo:QCH],
                                start=first_av, stop=(kc == nkc - 1),
                            )
                            first_av = False
                            kc += 1
                        if o["dbg_no_norm"]:
                            dnc = sb.tile(
                                [65, QCH], F32, name=f"dnc_{m}_{hh}_{qc}",
                                tag="ysT", bufs=2,
                            )
                            nc.vector.tensor_copy(dnc[:], yps[:])
                            nc.vector.tensor_copy(
                                yTn_sb[m][0:65, qc * QCH:(qc + 1) * QCH].bitcast(F32),
                                dnc[:],
                            )
                            continue
                        # normalize y into this head's half of the pair tiles
                        ysT = sb.tile(
                            [65, QCH], F32, name=f"ysT_{m}_{hh}_{qc}",
                            tag="ysT", bufs=2,
                        )
                        nc.vector.tensor_copy(ysT[:], yps[:])
                        for j in range(4):
                            ytp = psT.tile(
                                [128, 65], F32, name=f"ytp_{m}_{hh}_{qc}_{j}",
                                tag="tp",
                            )
                            nc.tensor.transpose(
                                ytp[0:128, 0:65],
                                ysT[0:65, j * 128:(j + 1) * 128],
                                ident_sb[0:65, 0:65],
                            )
                            rec = sb.tile(
                                [128, 1], F32, name=f"rec_{m}_{hh}_{qc}_{j}",
                                tag="rec", bufs=2,
                            )
                            nc.vector.reciprocal(rec[:], ytp[:, 64:65])
                            nc.vector.tensor_scalar_mul(
                                yq2[j][:, r:r + HD], ytp[:, 0:HD], rec[:]
                            )
                    # pair tiles -> transposed yT layout for the projection
                    for j in range(4 if not o["dbg_no_norm"] else 0):
                        tq = qc * 4 + j
                        ytb = psT.tile(
                            [128, 128], F32, name=f"ytb_{m}_{qc}_{j}", tag="tp"
                        )
                        nc.tensor.transpose(ytb[:], yq2[j][:], ident_sb[:])
                        nc.vector.tensor_copy(
                            yTn_sb[m][:, tq * 128:(tq + 1) * 128], ytb[:]
                        )

            # --- phase 3: partial projection out = yTn.T @ wp ---
            def emit_proj(qc):
                if o["dbg_no_proj"]:
                    return
                for tq in range(qc * 4, qc * 4 + 4):
                    osb = sb.tile(
                        [128, D], F32, name=f"osb_{tq}", tag="osb", bufs=3
                    )
                    for half in range(2):
                        pps = psA.tile(
                            [128, QCH], F32, name=f"pp_{tq}_{half}", tag="mm"
                        )
                        for i in range(4):
                            nc.tensor.matmul(
                                pps[:],
                                yTn_sb[i][:, tq * 128:(tq + 1) * 128],
                                wp_sb[i][:, half * QCH:(half + 1) * QCH],
                                start=(i == 0), stop=(i == 3),
                            )
                        nc.vector.tensor_copy(
                            osb[:, half * QCH:(half + 1) * QCH], pps[:]
                        )
                    nc.sync.dma_start(out_ap[tq * 128:(tq + 1) * 128, :], osb[:])

            # --- pipelined emission: qkv(t) -> attn(qc=t) -> proj(qc) ---
            def emit_all():
                emit_qkv(0)
                for qc in range(NTQ):
                    emit_attn(qc)
                    if qc + 1 < NTQ:
                        emit_qkv(qc + 1)
                    emit_proj(qc)

            if o["loop_n"] > 0:
                with tc.For_i(0, o["loop_n"], 1):
                    emit_all()
            else:
                emit_all()

    nc.compile()
    _CACHED_NC = nc
    return nc


def make_in_maps(x, w_attn, w_proj, mmdt="bf16"):
    import ml_dtypes

    mdt = ml_dtypes.bfloat16 if mmdt == "bf16" else np.float32
    x = np.asarray(x, dtype=np.float32)
    w_attn = np.asarray(w_attn, dtype=np.float32)
    w_proj = np.asarray(w_proj, dtype=np.float32)
    mask = np.triu(np.ones((128, 128), dtype=mdt))  # live iff tk <= tq
    ident = np.eye(128, dtype=np.float32)
    wkv = np.ascontiguousarray(w_attn[:, D:D + 2 * HD].astype(mdt))
    in_maps = []
    for c in range(N_CORES):
        b, g = c // 2, c % 2
        in_maps.append(
            {
                "xT": np.ascontiguousarray(x[b].T.astype(mdt)),
                "wq": np.ascontiguousarray(w_attn[:, g * GQ:(g + 1) * GQ].astype(mdt)),
                "wkv": wkv,
                "wp": np.ascontiguousarray(w_proj[g * GQ:(g + 1) * GQ, :].astype(mdt)),
                "mask": mask,
                "ident": ident,
            }
        )
    return in_maps


def kernel(x, w_attn, w_proj):
    nc = build_program()
    in_maps = make_in_maps(x, w_attn, w_proj, mmdt=DEFAULT_OPTS["mmdt"])
    res = bass_utils.run_bass_kernel_spmd(
        nc, in_maps, core_ids=list(range(N_CORES))
    )
    out = np.empty((B, T, D), dtype=np.float32)
    for b in range(B):
        out[b] = res.results[2 * b]["out"] + res.results[2 * b + 1]["out"]
    return out


# revision 29
# speedup vs baseline: 1.1764x; 1.1764x over previous
"""Causal MQA self-attention (B=4, T=2048, D=1024, H=16 q-heads, 1 shared KV head)
on 8 TRN2 NeuronCores.

Sharding: core c = (b, g) with b = c // 2 (batch), g = c % 2 (head group of 8
query heads). Tensor-parallel on c_attn q-output columns and c_proj rows;
shared K/V computed per core from replicated wkv columns. Each core emits a
partial [T, D] projection output; the host sums the two head-group partials
per batch.

Per-core math (all matmuls fp32r, PE-friendly layouts):
  qkvT = W.T @ x.T                      (x fed pre-transposed as xT [D, T])
  S^T[tk, tq] = k q^T (K=64)            (scores transposed: softmax dim on
                                         partitions so PV contracts on it)
  P^T = exp(S^T / 8) with causal block-skip + triangular mask on diagonal
  [y^T; sums] = [v | 1].T @ P^T         (row-sums ride along as output row 64)
  y_norm = y / sums                     (done in a small transposed layout)
  out_partial = y_norm.T @ wp_slice     ([T, D], accumulated over head dims)

Engines cannot move data across partitions, so: K is duplicated into both
partition halves (odd heads' q slices live at partition 64), V stays in the
upper half and transposes against the identity's lower-right block, and head
pairs share one [128, 128] normalized-y tile that transposes directly into
the paired yT layout the projection needs.
"""

import numpy as np

import concourse.bacc as bacc
import concourse.tile as tile
from concourse import mybir
from concourse import bass_utils

F32 = mybir.dt.float32
F32R = mybir.dt.float32r
BF16 = mybir.dt.bfloat16
EXP = mybir.ActivationFunctionType.Exp

B, T, D, H, HD = 4, 2048, 1024, 16, 64
N_CORES = 8
HPC = H // 2          # 8 query heads per core
GQ = HPC * HD         # 512 q columns per core
QCH = 512             # Tq chunk width
KCH = 128             # Tk chunk width
NTQ = T // QCH        # 4
NT128 = T // 128      # 16
ND = D // 128         # 8

_CACHED_NC = None

DEFAULT_OPTS = {
    "psA_bufs": 3,   # scores/qkv/proj PSUM tiles
    "psY_bufs": 2,   # PV-accumulator PSUM tiles
    "psT_bufs": 2,   # transpose PSUM tiles
    "ptf_bufs": 4,   # fully-live probability tiles
    "xt_bufs": 10,   # streamed xT tiles
    "av_trim": True,  # trim PV matmul to live columns on diagonal-band tiles
    # timing-only debug switches (produce wrong numerics):
    "dbg_no_exp": False,   # DVE copy instead of ACT exp
    "dbg_no_norm": False,  # skip the y-normalize/transpose chain
    "dbg_no_proj": False,  # skip the projection phase
    "dbg_no_qkv": False,   # skip the qkv matmuls
    "loop_n": 0,           # >0: wrap the whole pipeline in a HW loop (timing)
    "mmdt": "bf16",        # matmul operand dtype: "bf16" (2x PE) or "f32r"
    "wide_exp": False,     # pair fully-live tiles into [128,1024] PSUM + one exp
    "mm_mask": True,       # additive causal mask via PE identity-matmul (not DVE)
}


def build_program(opts=None, cache=True):
    """Build (and cache) the single-core Bass program shared by all 8 cores."""
    global _CACHED_NC
    if cache and opts is None and _CACHED_NC is not None:
        return _CACHED_NC
    o = dict(DEFAULT_OPTS)
    if opts:
        o.update(opts)
    if o["wide_exp"] and "psA_bufs" not in (opts or {}):
        o["psA_bufs"] = 2

    MDT = BF16 if o["mmdt"] == "bf16" else F32R

    nc = bacc.Bacc(
        "TRN2", target_bir_lowering=False, debug=False, num_devices=N_CORES
    )
    xT_ap = nc.dram_tensor("xT", [D, T], MDT, kind="ExternalInput").ap()
    wq_ap = nc.dram_tensor("wq", [D, GQ], MDT, kind="ExternalInput").ap()
    wkv_ap = nc.dram_tensor("wkv", [D, 2 * HD], MDT, kind="ExternalInput").ap()
    wp_ap = nc.dram_tensor("wp", [GQ, D], MDT, kind="ExternalInput").ap()
    mask_ap = nc.dram_tensor("mask", [128, 128], MDT, kind="ExternalInput").ap()
    ident_ap = nc.dram_tensor("ident", [128, 128], F32, kind="ExternalInput").ap()
    out_ap = nc.dram_tensor("out", [T, D], F32, kind="ExternalOutput").ap()

    with tile.TileContext(nc) as tc:
        with (
            tc.tile_pool(name="sb", bufs=1) as sb,
            tc.tile_pool(name="psA", bufs=o["psA_bufs"], space="PSUM") as psA,
            tc.tile_pool(name="psY", bufs=o["psY_bufs"], space="PSUM") as psY,
            tc.tile_pool(name="psT", bufs=o["psT_bufs"], space="PSUM") as psT,
        ):
            # --- persistent SBUF tensors ---
            wq_sb = [
                sb.tile([128, GQ], MDT, name=f"wq{d}", tag=f"wq{d}")
                for d in range(ND)
            ]
            wkv_sb = [
                sb.tile([128, 2 * HD], MDT, name=f"wkv{d}", tag=f"wkv{d}")
                for d in range(ND)
            ]
            wp_sb = [
                sb.tile([128, D], MDT, name=f"wp{i}", tag=f"wp{i}")
                for i in range(4)
            ]
            qT_sb = [
                sb.tile([128, T], MDT, name=f"qT{m}", tag=f"qT{m}")
                for m in range(4)
            ]
            kT_sb = sb.tile([128, T], MDT, name="kT", tag="kT")
            vT_sb = sb.tile([128, T], F32, name="vT", tag="vT")  # rows 64:128
            # v chunks + ones column: [128, 65] per 128-row T chunk
            vx_sb = sb.tile([128, NT128 * 65], MDT, name="vx", tag="vx")
            yTn_sb = [
                sb.tile([128, T], MDT, name=f"yTn{i}", tag=f"yTn{i}")
                for i in range(4)
            ]
            mask_sb = sb.tile([128, 128], MDT, name="mask_sb", tag="mask")
            ident_sb = sb.tile([128, 128], F32, name="ident_sb", tag="ident")
            identm_sb = sb.tile([128, 128], MDT, name="identm_sb", tag="identm")

            for d in range(ND):
                nc.sync.dma_start(wq_sb[d][:], wq_ap[d * 128:(d + 1) * 128, :])
                nc.sync.dma_start(wkv_sb[d][:], wkv_ap[d * 128:(d + 1) * 128, :])
            for i in range(4):
                nc.sync.dma_start(wp_sb[i][:], wp_ap[i * 128:(i + 1) * 128, :])
            nc.sync.dma_start(mask_sb[:], mask_ap[:])
            nc.sync.dma_start(ident_sb[:], ident_ap[:])
            nc.vector.tensor_copy(identm_sb[:], ident_sb[:])
            for t in range(NT128):
                ones_ap = vx_sb[:, t * 65 + 64:t * 65 + 65]
                nc.vector.memset(ones_ap.bitcast(F32) if MDT == F32R else ones_ap, 1.0)

            def emit_qkv(t):
                """QKV projection for T-chunk t: writes qT/kT (both halves)/vx
                columns [t*512, (t+1)*512)."""
                xts = []
                for d in range(ND):
                    xt = sb.tile(
                        [128, QCH], MDT, name=f"xt_{t}_{d}", tag="xt",
                        bufs=o["xt_bufs"],
                    )
                    nc.sync.dma_start(
                        xt[:], xT_ap[d * 128:(d + 1) * 128, t * QCH:(t + 1) * QCH]
                    )
                    xts.append(xt)
                for m in range(5):
                    if o["dbg_no_qkv"]:
                        break
                    ps = psA.tile(
                        [128, QCH], F32, name=f"qkvps_{t}_{m}", tag="mm"
                    )
                    for d in range(ND):
                        lhsT = (
                            wq_sb[d][:, m * 128:(m + 1) * 128]
                            if m < 4
                            else wkv_sb[d][:]
                        )
                        nc.tensor.matmul(
                            ps[:], lhsT, xts[d][:],
                            start=(d == 0), stop=(d == ND - 1),
                        )
                    if m < 4:
                        nc.vector.tensor_copy(
                            qT_sb[m][:, t * QCH:(t + 1) * QCH], ps[:]
                        )
                    else:
                        nc.vector.tensor_copy(
                            kT_sb[0:HD, t * QCH:(t + 1) * QCH], ps[0:HD, :]
                        )
                        nc.vector.tensor_copy(
                            vT_sb[HD:128, t * QCH:(t + 1) * QCH], ps[HD:128, :]
                        )
                # duplicate this k chunk into the lower half (SBUF->SBUF DMA)
                nc.sync.dma_start(
                    kT_sb[HD:128, t * QCH:(t + 1) * QCH],
                    kT_sb[0:HD, t * QCH:(t + 1) * QCH],
                )
                # v chunks transposed into [Tk, 64] layout (+ ones col set)
                for tt in range(t * 4, t * 4 + 4):
                    vtp = psT.tile([128, 65], F32, name=f"vtp_{tt}", tag="tp")
                    nc.tensor.transpose(
                        vtp[0:128, 0:HD],
                        vT_sb[HD:128, tt * 128:(tt + 1) * 128],
                        ident_sb[HD:128, HD:128],
                    )
                    nc.vector.tensor_copy(
                        vx_sb[:, tt * 65:tt * 65 + HD], vtp[0:128, 0:HD]
                    )

            # --- probability tiles, grouped by diagonal-offset class.
            # Dead columns are zeroed once; exp only ever writes live columns,
            # so the PV matmul can always read the full 512-wide tile. ---
            ptf_w = 2 * QCH if o["wide_exp"] else QCH
            pt_full = [
                sb.tile([128, ptf_w], MDT, name=f"ptf{i}", tag=f"ptf{i}")
                for i in range(o["ptf_bufs"])
            ]
            pt_diag = {
                off: [
                    sb.tile(
                        [128, QCH], MDT, name=f"ptd{off}_{i}", tag=f"ptd{off}_{i}"
                    )
                    for i in range(2)
                ]
                for off in (0, 128, 256, 384)
            }
            for off in (128, 256, 384):
                for til in pt_diag[off]:
                    dead = til[:, 0:off]
                    nc.vector.memset(dead.bitcast(F32) if MDT == F32R else dead, 0.0)

            # --- phase 2: attention, head pairs share one yq2 tile per chunk ---
            cnt_full = 0
            cnt_diag = {0: 0, 128: 0, 256: 0, 384: 0}

            def emit_attn(qc):
                nonlocal cnt_full
                for m in range(4):  # head pair (2m, 2m+1)
                    yq2 = [
                        sb.tile(
                            [128, 128], F32, name=f"yq2_{m}_{qc}_{j}",
                            tag=f"yq2_{j}", bufs=2,
                        )
                        for j in range(4)
                    ]
                    for hh in range(2):
                        r = hh * HD
                        yps = psY.tile(
                            [65, QCH], F32, name=f"y_{m}_{hh}_{qc}", tag="y"
                        )
                        nkc = (qc + 1) * (QCH // KCH)
                        n_full = nkc - 4  # tiles strictly below the diag band

                        def emit_scores(sps, s_lo, s_hi, kc):
                            nc.tensor.matmul(
                                sps[:, s_lo:s_hi],
                                kT_sb[r:r + HD, kc * KCH:(kc + 1) * KCH],
                                qT_sb[m][
                                    r:r + HD,
                                    qc * QCH + (s_lo % QCH):
                                    qc * QCH + (s_lo % QCH) + (s_hi - s_lo),
                                ],
                                start=True, stop=True,
                            )

                        def emit_exp(pt, sps, lo, hi):
                            if o["dbg_no_exp"]:
                                nc.vector.tensor_copy(pt[:, lo:hi], sps[:, lo:hi])
                            else:
                                nc.scalar.activation(
                                    pt[:, lo:hi], sps[:, lo:hi], EXP, scale=0.125
                                )

                        kc = 0
                        first_av = True
                        if o["wide_exp"]:
                            while kc + 1 < n_full:  # full tiles, in pairs
                                pt = pt_full[cnt_full % o["ptf_bufs"]]
                                cnt_full += 1
                                sps = psA.tile(
                                    [128, 2 * QCH], F32,
                                    name=f"s_{m}_{hh}_{qc}_{kc}", tag="mm",
                                )
                                emit_scores(sps, 0, QCH, kc)
                                emit_scores(sps, QCH, 2 * QCH, kc + 1)
                                emit_exp(pt, sps, 0, 2 * QCH)
                                nc.tensor.matmul(
                                    yps[:], vx_sb[:, kc * 65:kc * 65 + 65],
                                    pt[:, 0:QCH], start=first_av, stop=False,
                                )
                                nc.tensor.matmul(
                                    yps[:],
                                    vx_sb[:, (kc + 1) * 65:(kc + 1) * 65 + 65],
                                    pt[:, QCH:2 * QCH], start=False, stop=False,
                                )
                                first_av = False
                                kc += 2
                        while kc < nkc:
                            off = kc * KCH - qc * QCH
                            if off < 0:  # leftover single full tile
                                pt = pt_full[cnt_full % o["ptf_bufs"]]
                                cnt_full += 1
                                mm_lo = exp_lo = 0
                                diag = False
                            else:  # diagonal band tile
                                pt = pt_diag[off][cnt_diag[off] % 2]
                                cnt_diag[off] += 1
                                exp_lo = off
                                mm_lo = 256 if off == 384 else off
                                diag = True
                            sps = psA.tile(
                                [128, QCH], F32, name=f"s_{m}_{hh}_{qc}_{kc}",
                                tag="mm",
                            )
                            if diag and o["mm_mask"]:
                                nc.tensor.matmul(
                                    sps[:, mm_lo:QCH],
                                    kT_sb[r:r + HD, kc * KCH:(kc + 1) * KCH],
                                    qT_sb[m][
                                        r:r + HD,
                                        qc * QCH + mm_lo:(qc + 1) * QCH,
                                    ],
                                    start=True, stop=False,
                                )
                                nc.tensor.matmul(
                                    sps[:, exp_lo:exp_lo + 128],
                                    identm_sb[:], mask_sb[:],
                                    start=False, stop=True,
                                )
                            else:
                                emit_scores(sps, mm_lo, QCH, kc)
                            emit_exp(pt, sps, exp_lo, QCH)
                            if diag and not o["mm_mask"]:
                                nc.vector.tensor_mul(
                                    pt[:, exp_lo:exp_lo + 128],
                                    pt[:, exp_lo:exp_lo + 128],
                                    mask_sb[:],
                                )
                            av_lo = (exp_lo if o["av_trim"] else 0) if not first_av else 0
                            nc.tensor.matmul(
                                yps[:, av_lo:QCH],
                                vx_sb[:, kc * 65:kc * 65 + 65],
                                pt[:, av_lo:QCH],
                                start=first_av, stop=(kc == nkc - 1),
                            )
                            first_av = False
                            kc += 1
                        if o["dbg_no_norm"]:
                            dnc = sb.tile(
                                [65, QCH], F32, name=f"dnc_{m}_{hh}_{qc}",
                                tag="ysT", bufs=2,
                            )
                            nc.vector.tensor_copy(dnc[:], yps[:])
                            nc.vector.tensor_copy(
                                yTn_sb[m][0:65, qc * QCH:(qc + 1) * QCH].bitcast(F32),
                                dnc[:],
                            )
                            continue
                        # normalize y into this head's half of the pair tiles
                        ysT = sb.tile(
                            [65, QCH], F32, name=f"ysT_{m}_{hh}_{qc}",
                            tag="ysT", bufs=2,
                        )
                        nc.vector.tensor_copy(ysT[:], yps[:])
                        for j in range(4):
                            ytp = psT.tile(
                                [128, 65], F32, name=f"ytp_{m}_{hh}_{qc}_{j}",
                                tag="tp",
                            )
                            nc.tensor.transpose(
                                ytp[0:128, 0:65],
                                ysT[0:65, j * 128:(j + 1) * 128],
                                ident_sb[0:65, 0:65],
                            )
                            rec = sb.tile(
                                [128, 1], F32, name=f"rec_{m}_{hh}_{qc}_{j}",
                                tag="rec", bufs=2,
                            )
                            nc.vector.reciprocal(rec[:], ytp[:, 64:65])
                            nc.vector.tensor_scalar_mul(
                                yq2[j][:, r:r + HD], ytp[:, 0:HD], rec[:]
                            )
                    # pair tiles -> transposed yT layout for the projection
                    for j in range(4 if not o["dbg_no_norm"] else 0):
                        tq = qc * 4 + j
                        ytb = psT.tile(
                            [128, 128], F32, name=f"ytb_{m}_{qc}_{j}", tag="tp"
                        )
                        nc.tensor.transpose(ytb[:], yq2[j][:], ident_sb[:])
                        nc.vector.tensor_copy(
                            yTn_sb[m][:, tq * 128:(tq + 1) * 128], ytb[:]
                        )

            # --- phase 3: partial projection out = yTn.T @ wp ---
            def emit_proj(qc):
                if o["dbg_no_proj"]:
                    return
                for tq in range(qc * 4, qc * 4 + 4):
                    osb = sb.tile(
                        [128, D], F32, name=f"osb_{tq}", tag="osb", bufs=3
                    )
                    for half in range(2):
                        pps = psA.tile(
                            [128, QCH], F32, name=f"pp_{tq}_{half}", tag="mm"
                        )
                        for i in range(4):
                            nc.tensor.matmul(
                                pps[:],
                                yTn_sb[i][:, tq * 128:(tq + 1) * 128],
                                wp_sb[i][:, half * QCH:(half + 1) * QCH],
                                start=(i == 0), stop=(i == 3),
                            )
                        nc.vector.tensor_copy(
                            osb[:, half * QCH:(half + 1) * QCH], pps[:]
                        )
                    nc.sync.dma_start(out_ap[tq * 128:(tq + 1) * 128, :], osb[:])

            # --- pipelined emission: qkv(t) -> attn(qc=t) -> proj(qc) ---
            def emit_all():
                emit_qkv(0)
                for qc in range(NTQ):
                    emit_attn(qc)
                    if qc + 1 < NTQ:
                        emit_qkv(qc + 1)
                    emit_proj(qc)

            if o["loop_n"] > 0:
                with tc.For_i(0, o["loop_n"], 1):
                    emit_all()
            else:
                emit_all()

    nc.compile()
    _CACHED_NC = nc
    return nc


def make_in_maps(x, w_attn, w_proj, mmdt="bf16", mm_mask=True):
    import ml_dtypes

    mdt = ml_dtypes.bfloat16 if mmdt == "bf16" else np.float32
    x = np.asarray(x, dtype=np.float32)
    w_attn = np.asarray(w_attn, dtype=np.float32)
    w_proj = np.asarray(w_proj, dtype=np.float32)
    live = np.triu(np.ones((128, 128), dtype=np.float32))  # live iff tk <= tq
    if mm_mask:  # additive: 0 on live, large negative on dead (exp -> 0)
        mask = np.where(live > 0, 0.0, -2048.0).astype(mdt)
    else:  # multiplicative 0/1
        mask = live.astype(mdt)
    ident = np.eye(128, dtype=np.float32)
    wkv = np.ascontiguousarray(w_attn[:, D:D + 2 * HD].astype(mdt))
    in_maps = []
    for c in range(N_CORES):
        b, g = c // 2, c % 2
        in_maps.append(
            {
                "xT": np.ascontiguousarray(x[b].T.astype(mdt)),
                "wq": np.ascontiguousarray(w_attn[:, g * GQ:(g + 1) * GQ].astype(mdt)),
                "wkv": wkv,
                "wp": np.ascontiguousarray(w_proj[g * GQ:(g + 1) * GQ, :].astype(mdt)),
                "mask": mask,
                "ident": ident,
            }
        )
    return in_maps


def kernel(x, w_attn, w_proj):
    nc = build_program()
    in_maps = make_in_maps(
        x, w_attn, w_proj,
        mmdt=DEFAULT_OPTS["mmdt"], mm_mask=DEFAULT_OPTS["mm_mask"],
    )
    res = bass_utils.run_bass_kernel_spmd(
        nc, in_maps, core_ids=list(range(N_CORES))
    )
    out = np.empty((B, T, D), dtype=np.float32)
    for b in range(B):
        out[b] = res.results[2 * b]["out"] + res.results[2 * b + 1]["out"]
    return out
